# revision 1
# baseline (speedup 1.0000x reference)
"""Causal self-attention (B=2, T=2048, C=1024, 16 heads) on 8 trn2 cores.

Sharding: 2 batches x 4 head-groups (4 heads each). Per core, t-chunk-major
pipeline: stream x columns per 512-wide chunk, project qkv for the chunk,
then run causal attention for the chunk immediately (causality: chunk c only
attends to k/v tiles <= c). Scores stay transposed [tk, tq]; y^T accumulates
in psum with a ones-column denominator row; normalization via K=1 broadcast
matmul + fast reciprocal. y^T is AllGathered across the 4 cores of each
batch per chunk (overlapped with the next chunk's attention), followed by a
transposed column-slice output projection per chunk. Host re-transposes and
concatenates the 8 [256, T] slices.
"""
import numpy as np
import ml_dtypes

import concourse.bacc as bacc
import concourse.mybir as mybir
import concourse.tile as tile
from concourse.bass_utils import run_bass_kernel_spmd

F32 = mybir.dt.float32
F32R = mybir.dt.float32r
BF16 = mybir.dt.bfloat16
EXP = mybir.ActivationFunctionType.Exp

B, T, C = 2, 2048, 1024
NH, HD = 16, 64
NCORES = 8
NG = 4            # head groups (tensor-parallel within a batch)
GC = 256          # features per group (4 heads * 64)
NFT = C // 128    # 8 feature tiles
NTT = T // 128    # 16 t tiles
NCH = T // 512    # 4 tq chunks

_nc_cache = {}


def build_nc():
    nc = bacc.Bacc("TRN2", target_bir_lowering=False, debug=False, num_devices=NCORES)
    xT = nc.dram_tensor("xT", [C, T], F32R, kind="ExternalInput")
    wq = nc.dram_tensor("wq", [C, GC], F32R, kind="ExternalInput")
    wk = nc.dram_tensor("wk", [C, GC], F32R, kind="ExternalInput")
    wv = nc.dram_tensor("wv", [C, GC], F32R, kind="ExternalInput")
    wpr = nc.dram_tensor("wpr", [GC, C], F32R, kind="ExternalInput")
    tri = nc.dram_tensor("tri", [128, 128], F32R, kind="ExternalInput")
    ones = nc.dram_tensor("ones", [128, 64], F32R, kind="ExternalInput")
    outP = nc.dram_tensor("outP", [C, T], F32, kind="ExternalOutput")

    with tile.TileContext(nc) as tc:
        with (
            tc.tile_pool(name="xc", bufs=1) as xcp,        # streamed x chunks
            tc.tile_pool(name="wpool", bufs=1) as wpool,
            tc.tile_pool(name="qk", bufs=1) as qkpool,
            tc.tile_pool(name="vpool", bufs=1) as vpool,
            tc.tile_pool(name="work", bufs=1) as work,
            tc.tile_pool(name="ytpool", bufs=1) as ytpool,
            tc.tile_pool(name="dram", bufs=1, space="DRAM") as dram,
            tc.tile_pool(name="psum", bufs=1, space="PSUM") as ps,
        ):
            # ---------------- loads: tiny consts, then x chunk0 + wq interleaved ----------------
            tri_sb = wpool.tile([128, 128], F32R, name="tri_sb")
            nc.sync.dma_start(tri_sb[:], tri[:])
            ones_sb = wpool.tile([128, 64], F32R, name="ones_sb")
            nc.sync.dma_start(ones_sb[:], ones[:])

            xc_tiles = {}

            def load_xc(c):
                xc = [
                    xcp.tile([128, 512], F32R, tag="xc", bufs=16, name=f"xc{c}_{i}")
                    for i in range(NFT)
                ]
                for i in range(NFT):
                    nc.sync.dma_start(
                        xc[i][:], xT[128 * i : 128 * (i + 1), 512 * c : 512 * (c + 1)]
                    )
                    if c == 0:
                        nc.sync.dma_start(wqt[i][:], wq[128 * i : 128 * (i + 1), :])
                xc_tiles[c] = xc

            wqt = [wpool.tile([128, GC], F32R, tag="wq", bufs=8, name=f"wqt{i}") for i in range(NFT)]
            wkt = [wpool.tile([128, GC], F32R, tag="wk", bufs=8, name=f"wkt{i}") for i in range(NFT)]
            wvt = [wpool.tile([128, GC], F32R, tag="wv", bufs=8, name=f"wvt{i}") for i in range(NFT)]
            wprt = [wpool.tile([128, C], F32R, tag="wpr", bufs=2, name=f"wprt{f}") for f in range(2)]

            load_xc(0)
            for i in range(NFT):
                nc.sync.dma_start(wkt[i][:], wk[128 * i : 128 * (i + 1), :])
            for i in range(NFT):
                nc.sync.dma_start(wvt[i][:], wv[128 * i : 128 * (i + 1), :])
            for f in range(2):
                nc.sync.dma_start(wprt[f][:], wpr[128 * f : 128 * (f + 1), :])

            # persistent per-core tensors
            qT = [qkpool.tile([128, T], F32R, tag="qT", bufs=2, name=f"qT{p}") for p in range(2)]
            kT = [qkpool.tile([128, T], F32R, tag="kT", bufs=2, name=f"kT{p}") for p in range(2)]
            vb = [vpool.tile([128, 260], F32R, tag="v", bufs=NTT, name=f"vb{tt}") for tt in range(NTT)]
            yT_sb = [
                ytpool.tile([64, T], F32R, tag="yt", bufs=4, name=f"yTsb{ph}")
                for ph in range(4)
            ]
            def proj_partial(c):
                # pack the 4 heads' normalized y^T for chunk c into [128, 512]
                # tiles (cross-partition move => DMA), then project against the
                # row-slice of W_proj; host sums partials across the 4 cores.
                ypk = [
                    work.tile([128, 512], F32R, tag="ypk", bufs=4, name=f"ypk{c}_{f}")
                    for f in range(2)
                ]
                for pp in range(2):
                    for h in range(2):
                        nc.sync.dma_start(
                            ypk[pp][64 * h : 64 * (h + 1), :],
                            yT_sb[2 * pp + h][:, 512 * c : 512 * (c + 1)],
                        )
                for u in range(NFT):
                    opp = ps.tile([128, 512], F32, tag="mix", bufs=2, name=f"opp{c}{u}")
                    for f in range(2):
                        nc.tensor.matmul(
                            opp[:],
                            wprt[f][:, 128 * u : 128 * (u + 1)],
                            ypk[f][:],
                            start=(f == 0),
                            stop=(f == 1),
                        )
                    osb = work.tile([128, 512], F32, tag="osb", bufs=3, name=f"osb{c}{u}")
                    nc.vector.tensor_copy(osb[:], opp[:])
                    nc.sync.dma_start(
                        outP[128 * u : 128 * (u + 1), 512 * c : 512 * (c + 1)], osb[:]
                    )

            pending_norm = []

            def flush_norms():
                for (p_, c_, h_, yrw) in pending_norm:
                    bc = ps.tile([64, 512], F32, tag="mix", bufs=2, name=f"bc{p_}{c_}{h_}")
                    nc.tensor.matmul(
                        bc[:], ones_sb[64:65, :], yrw[64:65, :], start=True, stop=True
                    )
                    rcp = work.tile([64, 512], F32, tag="rcp", bufs=2, name=f"rcp{p_}{c_}{h_}")
                    nc.vector.reciprocal_approx_fast(rcp[:], bc[:])
                    nc.vector.tensor_mul(
                        yT_sb[2 * p_ + h_][:, 512 * c_ : 512 * (c_ + 1)],
                        yrw[0:64, :],
                        rcp[:],
                    )
                pending_norm.clear()

            # ---------------- per-chunk pipeline ----------------
            for c in range(NCH):
                xc = xc_tiles[c]
                # qkv for this chunk
                for p in range(2):
                    qps = ps.tile([128, 512], F32, tag="mix", bufs=2, name=f"qps{p}_{c}")
                    for i in range(NFT):
                        nc.tensor.matmul(
                            qps[:],
                            wqt[i][:, 128 * p : 128 * (p + 1)],
                            xc[i][:],
                            start=(i == 0),
                            stop=(i == NFT - 1),
                        )
                    nc.vector.tensor_copy(qT[p][:, 512 * c : 512 * (c + 1)], qps[:])
                    kps = ps.tile([128, 512], F32, tag="mix", bufs=2, name=f"kps{p}_{c}")
                    for i in range(NFT):
                        nc.tensor.matmul(
                            kps[:],
                            wkt[i][:, 128 * p : 128 * (p + 1)],
                            xc[i][:],
                            start=(i == 0),
                            stop=(i == NFT - 1),
                        )
                    nc.vector.tensor_copy(kT[p][:, 512 * c : 512 * (c + 1)], kps[:])
                for ttl in range(4):
                    tt = 4 * c + ttl
                    vps = ps.tile([128, 256], F32, tag="mix", bufs=2, name=f"vps{tt}")
                    for i in range(NFT):
                        nc.tensor.matmul(
                            vps[:],
                            xc[i][:, 128 * ttl : 128 * (ttl + 1)],
                            wvt[i][:],
                            start=(i == 0),
                            stop=(i == NFT - 1),
                        )
                    nc.sync.dma_start(
                        vb[tt][:].rearrange("p (s c) -> p s c", s=4)[:, :, 64:65],
                        ones_sb[:, 0:4].rearrange("p (s o) -> p s o", o=1),
                    )
                    nc.vector.tensor_copy(
                        vb[tt][:].rearrange("p (s c) -> p s c", s=4)[:, :, 0:64],
                        vps[:].rearrange("p (s c) -> p s c", s=4),
                    )
                # prefetch next chunk's x right behind this chunk's compute wave
                if c + 1 < NCH:
                    load_xc(c + 1)

                flush_norms()
                if c >= 1:
                    proj_partial(c - 1)

                # attention for this chunk; j-loop software-pipelined one
                # stage so PE runs scores(j+1) while ACT computes exp(j)
                for p in range(2):
                    yta = [
                        ps.tile([65, 512], F32, tag=f"yta{h}", bufs=1, name=f"yta{p}{c}{h}")
                        for h in range(2)
                    ]

                    def scores_exp(j):
                        d = j - 4 * c
                        off = 128 * max(d, 0)
                        sps = ps.tile([128, 1024], F32, tag="sps", bufs=2, name=f"sps{p}{c}{j}")
                        for h in range(2):
                            nc.tensor.matmul(
                                sps[:, 512 * h + off : 512 * (h + 1)],
                                kT[p][64 * h : 64 * (h + 1), 128 * j : 128 * (j + 1)],
                                qT[p][64 * h : 64 * (h + 1), 512 * c + off : 512 * (c + 1)],
                                start=True,
                                stop=True,
                            )
                        es = work.tile([128, 1024], F32R, tag="es", bufs=4, name=f"es{p}{c}{j}")
                        nc.scalar.activation(
                            es[:].rearrange("p (g n) -> p g n", g=2)[:, :, off:512],
                            sps[:].rearrange("p (g n) -> p g n", g=2)[:, :, off:512],
                            EXP,
                            scale=0.125,
                        )
                        if d >= 0:
                            for h in range(2):
                                nc.vector.tensor_mul(
                                    es[:, 512 * h + off : 512 * h + off + 128],
                                    es[:, 512 * h + off : 512 * h + off + 128],
                                    tri_sb[:],
                                )
                        return es

                    def av(j, es):
                        d = j - 4 * c
                        off = 128 * max(d, 0)
                        for h in range(2):
                            hs = 2 * p + h
                            nc.tensor.matmul(
                                yta[h][:, off:512],
                                vb[j][:, 65 * hs : 65 * hs + 65],
                                es[:, 512 * h + off : 512 * (h + 1)],
                                start=(j == 0),
                                stop=(j == 4 * c + 3),
                            )

                    prev = None
                    for j in range(4 * c + 4):
                        es = scores_exp(j)
                        if prev is not None:
                            av(*prev)
                        prev = (j, es)
                    av(*prev)
                    # evacuate yta now (releases psum); defer the normalize
                    # (bc matmul + recip + mul) so PE is not stalled here
                    for h in range(2):
                        yrw = work.tile([65, 512], F32R, tag="yrw", bufs=6, name=f"yrw{p}{c}{h}")
                        nc.vector.tensor_copy(yrw[:], yta[h][:])
                        pending_norm.append((p, c, h, yrw))

            flush_norms()
            proj_partial(NCH - 1)

    nc.compile()
    return nc


def _get_nc():
    if "nc" not in _nc_cache:
        _nc_cache["nc"] = build_nc()
    return _nc_cache["nc"]


def _in_maps(x, W_attn, W_proj):
    tri = np.triu(np.ones((128, 128), np.float32))
    ones = np.ones((128, 64), np.float32)
    maps = []
    for core in range(NCORES):
        b, g = core // NG, core % NG
        lo = g * GC
        maps.append(
            {
                "xT": np.ascontiguousarray(x[b].T),
                "wq": np.ascontiguousarray(W_attn[:, lo : lo + GC]),
                "wk": np.ascontiguousarray(W_attn[:, C + lo : C + lo + GC]),
                "wv": np.ascontiguousarray(W_attn[:, 2 * C + lo : 2 * C + lo + GC]),
                "wpr": np.ascontiguousarray(W_proj[lo : lo + GC, :]),
                "tri": tri,
                "ones": ones,
            }
        )
    return maps


def kernel(x, W_attn, W_proj, **run_kwargs):
    x = np.asarray(x, np.float32)
    W_attn = np.asarray(W_attn, np.float32)
    W_proj = np.asarray(W_proj, np.float32)
    nc = _get_nc()
    res = run_bass_kernel_spmd(
        nc, _in_maps(x, W_attn, W_proj), core_ids=list(range(NCORES)), **run_kwargs
    )
    out = np.empty((B, T, C), np.float32)
    for b in range(B):
        acc = res.results[NG * b]["outP"].copy()
        for g in range(1, NG):
            acc += res.results[NG * b + g]["outP"]
        out[b] = acc.T
    if run_kwargs:
        kernel.last_result = res
    return out



# revision 7
# speedup vs baseline: 1.1986x; 1.1986x over previous
"""Causal self-attention (B=2, T=2048, C=1024, 16 heads) on 8 trn2 cores.

Sharding: 2 batches x 4 head-groups (4 heads each per core). All matmul
operands bf16 (host-converted); psum f32.

Per core, t-chunk-major pipeline over 4 chunks of 512 queries:
  - qkv projection per chunk (q/k transposed [feat, t] per-chunk tiles,
    v in [t, feat+ones] layout for the denominator trick).
  - attention per chunk: scores kept transposed [t_k, t_q] in psum, exp on
    ACT -> bf16 sbuf, causal diagonal masked via tri multiply (DVE).
  - av uses es as the *stationary* operand: out y[t_q, 65] accumulates over
    t_k tiles at 65 rows/pass (4x fewer PE rows than the y^T layout).
    Column 64 accumulates the softmax denominator via the ones column of v.
  - normalize with per-partition reciprocal + tensor_scalar (DVE), then PE
    transpose back to [feat, t_q] for the output projection.
  - output projection per chunk, evacuated by GPSIMD, stored bf16.

qkv(c+1) and proj(c-1) matmuls are interleaved as fillers into the
attention(c) instruction stream so the PE stays busy while ACT computes exp.
DMAs are batched (3D access patterns) to keep HWDGE occupancy low; loads and
stores share the SP queue but prefetch runs a chunk ahead.

Host re-transposes, accumulates the 4 partial [C, T] projections per batch.
"""
import numpy as np
import ml_dtypes

import concourse.bacc as bacc
import concourse.mybir as mybir
import concourse.tile as tile
from concourse.bass_utils import run_bass_kernel_spmd

F32 = mybir.dt.float32
BF16 = mybir.dt.bfloat16
EXP = mybir.ActivationFunctionType.Exp

B, T, C = 2, 2048, 1024
NH, HD = 16, 64
NCORES = 8
NG = 4            # head groups (tensor-parallel within a batch)
GC = 256          # features per group (4 heads * 64)
NFT = C // 128    # 8 feature tiles
NCH = T // 512    # 4 tq chunks

_nc_cache = {}


def build_nc():
    nc = bacc.Bacc("TRN2", target_bir_lowering=False, debug=False, num_devices=NCORES)
    xT = nc.dram_tensor("xT", [C, T], BF16, kind="ExternalInput")
    wq = nc.dram_tensor("wq", [C, GC], BF16, kind="ExternalInput")
    wk = nc.dram_tensor("wk", [C, GC], BF16, kind="ExternalInput")
    wv = nc.dram_tensor("wv", [C, GC], BF16, kind="ExternalInput")
    wpr = nc.dram_tensor("wpr", [GC, C], BF16, kind="ExternalInput")
    tri = nc.dram_tensor("tri", [128, 128], BF16, kind="ExternalInput")
    ident = nc.dram_tensor("ident", [128, 128], F32, kind="ExternalInput")
    outP = nc.dram_tensor("outP", [C, T], BF16, kind="ExternalOutput")

    xTv = xT[:].rearrange("(i p) t -> p i t", p=128)      # [128, 8, 2048]
    wqv = wq[:].rearrange("(i p) f -> p i f", p=128)      # [128, 8, 256]
    wkv = wk[:].rearrange("(i p) f -> p i f", p=128)
    wvv = wv[:].rearrange("(i p) f -> p i f", p=128)
    wprv = wpr[:].rearrange("(f p) c -> p f c", p=128)    # [128, 2, 1024]
    outv = outP[:].rearrange("(u r) t -> r u t", r=128)   # [128, 8, 2048]

    with tile.TileContext(nc) as tc:
        with (
            tc.tile_pool(name="wts", bufs=1) as wts,
            tc.tile_pool(name="xcp", bufs=1) as xcp,
            tc.tile_pool(name="qkp", bufs=1) as qkp,
            tc.tile_pool(name="vp", bufs=1) as vp,
            tc.tile_pool(name="esp", bufs=1) as esp,
            tc.tile_pool(name="ywp", bufs=1) as ywp,
            tc.tile_pool(name="outp", bufs=1) as outp,
            tc.tile_pool(name="ps", bufs=1, space="PSUM") as ps,
        ):
            tri_sb = wts.tile([128, 128], BF16, name="tri_sb")
            ident_sb = wts.tile([128, 128], F32, name="ident_sb")
            wq_sb = wts.tile([128, 2048], BF16, name="wq_sb")
            wk_sb = wts.tile([128, 2048], BF16, name="wk_sb")
            wv_sb = wts.tile([128, 2048], BF16, name="wv_sb")
            wpr_sb = wts.tile([128, 2048], BF16, name="wpr_sb")

            xc_tiles = {}

            def load_xc(c, split=False):
                xc = xcp.tile([128, 4096], BF16, tag="xc", bufs=3, name=f"xc{c}")
                xcv = xc[:].rearrange("p (i t) -> p i t", i=NFT)
                src = xTv[:, :, 512 * c : 512 * (c + 1)]
                if split:
                    # interleave per-feature-block with wq so the first qkv
                    # matmuls can start as early as possible
                    wq_sbv = wq_sb[:].rearrange("p (i f) -> p i f", i=NFT)
                    for i in range(NFT):
                        nc.sync.dma_start(wq_sbv[:, i : i + 1, :], wqv[:, i : i + 1, :])
                        nc.sync.dma_start(xcv[:, i : i + 1, :], src[:, i : i + 1, :])
                else:
                    nc.sync.dma_start(xcv, src)
                xc_tiles[c] = xc

            # ---------------- initial loads (SP queue) ----------------
            nc.sync.dma_start(tri_sb[:], tri[:])
            nc.sync.dma_start(ident_sb[:], ident[:])
            load_xc(0, split=True)
            nc.sync.dma_start(
                wk_sb[:].rearrange("p (i f) -> p i f", i=NFT), wkv
            )
            nc.sync.dma_start(
                wv_sb[:].rearrange("p (i f) -> p i f", i=NFT), wvv
            )
            load_xc(1)
            nc.sync.dma_start(
                wpr_sb[:].rearrange("p (f c) -> p f c", f=2), wprv
            )
            load_xc(2)
            load_xc(3)

            qT, kT, vb = {}, {}, {}
            ypk_tiles = {}

            def qkv_closures(c):
                """Emit chunk c's qkv projection as a list of closures
                (~2 matmuls each) usable as PE fillers."""
                xc = xc_tiles[c]
                cls = []
                for kind, wsb, dstmap in (("q", wq_sb, qT), ("k", wk_sb, kT)):
                    for p in range(2):
                        dst = qkp.tile(
                            [128, 512], BF16, tag="qkT", bufs=16,
                            name=f"{kind}T{c}_{p}",
                        )
                        dstmap[(c, p)] = dst
                        st = {}
                        for i0 in range(0, NFT, 2):
                            def go(kind=kind, wsb=wsb, p=p, dst=dst, st=st, i0=i0, c=c):
                                if i0 == 0:
                                    st["ps"] = ps.tile(
                                        [128, 512], F32, tag="mix", bufs=2,
                                        name=f"{kind}ps{c}_{p}",
                                    )
                                pst = st["ps"]
                                for i in (i0, i0 + 1):
                                    nc.tensor.matmul(
                                        pst[:],
                                        wsb[:, 256 * i + 128 * p : 256 * i + 128 * (p + 1)],
                                        xc[:, 512 * i : 512 * (i + 1)],
                                        start=(i == 0),
                                        stop=(i == NFT - 1),
                                    )
                                if i0 == NFT - 2:
                                    nc.vector.tensor_copy(dst[:], pst[:])
                            cls.append(go)
                for ttl in range(4):
                    tt = 4 * c + ttl
                    dst = vp.tile([128, 260], BF16, tag="vb", bufs=16, name=f"vb{tt}")
                    vb[tt] = dst
                    st = {}
                    for i0 in range(0, NFT, 2):
                        def go(ttl=ttl, tt=tt, dst=dst, st=st, i0=i0, c=c, xc=xc):
                            if i0 == 0:
                                st["ps"] = ps.tile(
                                    [128, 256], F32, tag="mix", bufs=2,
                                    name=f"vps{tt}",
                                )
                            pst = st["ps"]
                            for i in (i0, i0 + 1):
                                nc.tensor.matmul(
                                    pst[:],
                                    xc[:, 512 * i + 128 * ttl : 512 * i + 128 * (ttl + 1)],
                                    wv_sb[:, 256 * i : 256 * (i + 1)],
                                    start=(i == 0),
                                    stop=(i == NFT - 1),
                                )
                            if i0 == NFT - 2:
                                dv = dst[:].rearrange("p (s c) -> p s c", s=4)
                                nc.vector.tensor_copy(
                                    dv[:, :, 0:64],
                                    pst[:].rearrange("p (s c) -> p s c", s=4),
                                )
                                nc.vector.memset(dv[:, :, 64:65], 1.0)
                        cls.append(go)
                return cls

            def proj_closures(c):
                """Output projection for chunk c (ypk must be complete)."""
                cls = []
                osb = outp.tile([128, 4096], BF16, tag="osb", bufs=2, name=f"osb{c}")
                for u in range(NFT):
                    def go(u=u, c=c, osb=osb):
                        op = ps.tile(
                            [128, 512], F32, tag="mix", bufs=2, name=f"opp{c}_{u}"
                        )
                        for f in range(2):
                            nc.tensor.matmul(
                                op[:],
                                wpr_sb[:, 1024 * f + 128 * u : 1024 * f + 128 * (u + 1)],
                                ypk_tiles[c][f][:],
                                start=(f == 0),
                                stop=(f == 1),
                            )
                        nc.vector.tensor_copy(osb[:, 512 * u : 512 * (u + 1)], op[:])
                    cls.append(go)

                def store(c=c, osb=osb):
                    nc.sync.dma_start(
                        outv[:, :, 512 * c : 512 * (c + 1)],
                        osb[:].rearrange("p (u t) -> p u t", u=NFT),
                    )
                cls.append(store)
                return cls

            def build_fillers(c):
                a = proj_closures(c - 1) if c >= 1 else []
                b = qkv_closures(c + 1) if c + 1 < NCH else []
                out = []
                ia = ib = 0
                while ia < len(a) or ib < len(b):
                    for _ in range(4):
                        if ib < len(b):
                            out.append(b[ib]); ib += 1
                    if ia < len(a):
                        out.append(a[ia]); ia += 1
                return out

            def attention(c, fillers):
                nf = len(fillers)
                state = {"emitted": 0, "step": 0}
                total_steps = 2 * (4 * c + 4)

                def pump_to(tgt):
                    while state["emitted"] < min(nf, tgt):
                        fillers[state["emitted"]]()
                        state["emitted"] += 1

                def pump_frac():
                    state["step"] += 1
                    pump_to(int(round(nf * state["step"] / total_steps)))

                for p in range(2):
                    # 8 accumulation groups packed into 2 psum banks. The
                    # interpreter's start_tensor_calc wipes the whole 2KB
                    # zero-region, so only the first group per bank carries
                    # start=True; later groups' bytes are already pending-zero
                    # and get replace semantics on their first write. Group 7
                    # sits at col 512 (bank 1) so no group straddles banks.
                    yac = ps.tile([128, 577], F32, tag="yac", bufs=1, name=f"yac{c}_{p}")
                    ysb = {}
                    nj = 4 * c + 4
                    esq = []

                    def scores_exp(j, p=p):
                        d = j - 4 * c
                        off = 128 * max(d, 0)
                        sps = ps.tile(
                            [128, 1024], F32, tag="sps", bufs=2, name=f"sps{c}{p}{j}"
                        )
                        jc, jj = j // 4, j % 4
                        for h in range(2):
                            nc.tensor.matmul(
                                sps[:, 512 * h + off : 512 * (h + 1)],
                                kT[(jc, p)][64 * h : 64 * (h + 1), 128 * jj : 128 * (jj + 1)],
                                qT[(c, p)][64 * h : 64 * (h + 1), off:512],
                                start=True,
                                stop=True,
                            )
                        es = esp.tile(
                            [128, 1024], BF16, tag="es", bufs=5, name=f"es{c}{p}{j}"
                        )
                        nc.scalar.activation(
                            es[:].rearrange("p (g n) -> p g n", g=2)[:, :, off:512],
                            sps[:].rearrange("p (g n) -> p g n", g=2)[:, :, off:512],
                            EXP,
                            scale=0.125,
                        )
                        if d >= 0:
                            for h in range(2):
                                nc.vector.tensor_mul(
                                    es[:, 512 * h + off : 512 * h + off + 128],
                                    es[:, 512 * h + off : 512 * h + off + 128],
                                    tri_sb[:],
                                )
                        return es

                    def av(j, es, p=p, yac=yac):
                        d = j - 4 * c
                        for q0 in range(max(d, 0), 4):
                            for h in range(2):
                                hs = 2 * p + h
                                g = 2 * q0 + h
                                base = 65 * g if g < 7 else 512
                                nc.tensor.matmul(
                                    yac[:, base : base + 65],
                                    es[:, 512 * h + 128 * q0 : 512 * h + 128 * (q0 + 1)],
                                    vb[j][:, 65 * hs : 65 * hs + 65],
                                    start=(j == 0 and g in (0, 7)),
                                    stop=(j == 4 * c + q0),
                                    skip_group_check=True,
                                )

                    def normalize(q0, p=p, yac=yac, ysb=ysb):
                        yt = ywp.tile(
                            [128, 128], F32, tag="ysb", bufs=4, name=f"ysb{c}{p}{q0}"
                        )
                        for h in range(2):
                            g = 2 * q0 + h
                            base = 65 * g if g < 7 else 512
                            rc = ywp.tile(
                                [128, 1], F32, tag="rcp", bufs=8,
                                name=f"rcp{c}{p}{q0}{h}",
                            )
                            nc.vector.reciprocal_approx_fast(
                                rc[:], yac[:, base + 64 : base + 65]
                            )
                            nc.vector.tensor_scalar_mul(
                                yt[:, 64 * h : 64 * (h + 1)],
                                yac[:, base : base + 64],
                                rc[:],
                            )
                        ysb[q0] = yt

                    def do_av():
                        j_, es_ = esq.pop(0)
                        av(j_, es_)
                        if j_ >= 4 * c:
                            normalize(j_ - 4 * c)

                    for j in range(nj):
                        esq.append((j, scores_exp(j)))
                        pump_frac()
                        if len(esq) > 2:
                            do_av()
                    while esq:
                        do_av()

                    # transpose normalized y back to [feat, t] for the projection
                    mixT = ps.tile([128, 512], F32, tag="mix", bufs=2, name=f"yT{c}_{p}")
                    for q0 in range(4):
                        # one psum group for all 4 transposes: start=True would
                        # wipe the shared 2KB zero-region of the earlier ones
                        nc.tensor.matmul(
                            mixT[:, 128 * q0 : 128 * (q0 + 1)], ysb[q0][:], ident_sb[:],
                            is_transpose=True, start=(q0 == 0), stop=(q0 == 3),
                            skip_group_check=True,
                        )
                    yp = ywp.tile([128, 512], BF16, tag="ypk", bufs=4, name=f"ypk{c}_{p}")
                    nc.vector.tensor_copy(yp[:], mixT[:])
                    ypk_tiles.setdefault(c, {})[p] = yp

                pump_to(nf)

            # ---------------- main schedule ----------------
            for cl in qkv_closures(0):
                cl()
            for c in range(NCH):
                attention(c, build_fillers(c))
            for cl in proj_closures(NCH - 1):
                cl()

    nc.compile()
    return nc


def _get_nc():
    if "nc" not in _nc_cache:
        _nc_cache["nc"] = build_nc()
    return _nc_cache["nc"]


def _in_maps(x, W_attn, W_proj):
    bf = ml_dtypes.bfloat16
    tri = np.triu(np.ones((128, 128), np.float32)).astype(bf)
    ident = np.eye(128, dtype=np.float32)
    maps = []
    for core in range(NCORES):
        b, g = core // NG, core % NG
        lo = g * GC
        maps.append(
            {
                "xT": np.ascontiguousarray(x[b].T).astype(bf),
                "wq": np.ascontiguousarray(W_attn[:, lo : lo + GC]).astype(bf),
                "wk": np.ascontiguousarray(W_attn[:, C + lo : C + lo + GC]).astype(bf),
                "wv": np.ascontiguousarray(W_attn[:, 2 * C + lo : 2 * C + lo + GC]).astype(bf),
                "wpr": np.ascontiguousarray(W_proj[lo : lo + GC, :]).astype(bf),
                "tri": tri,
                "ident": ident,
            }
        )
    return maps


def kernel(x, W_attn, W_proj, **run_kwargs):
    x = np.asarray(x, np.float32)
    W_attn = np.asarray(W_attn, np.float32)
    W_proj = np.asarray(W_proj, np.float32)
    nc = _get_nc()
    res = run_bass_kernel_spmd(
        nc, _in_maps(x, W_attn, W_proj), core_ids=list(range(NCORES)), **run_kwargs
    )
    out = np.empty((B, T, C), np.float32)
    for b in range(B):
        acc = res.results[NG * b]["outP"].astype(np.float32)
        for g in range(1, NG):
            acc += res.results[NG * b + g]["outP"].astype(np.float32)
        out[b] = acc.T
    if run_kwargs:
        kernel.last_result = res
    return out


# revision 10
# speedup vs baseline: 1.2216x; 1.0192x over previous
"""Causal self-attention (B=2, T=2048, C=1024, 16 heads) on 8 trn2 cores.

Sharding: 2 batches x 4 head-groups (4 heads each per core). All matmul
operands bf16 (host-converted); psum f32.

Per core, a single software-pipelined stream over (chunk, p-half, key-tile)
units:
  - qkv projection per 512-query chunk (q/k transposed [feat, t] per-chunk
    tiles, v in [t, feat+ones] layout for the denominator trick).
  - scores kept transposed [t_k, t_q] in psum, exp on ACT -> bf16 sbuf,
    causal diagonal masked via tri multiply (DVE).
  - av uses es as the *stationary* operand: out y[t_q, 65] accumulates over
    t_k tiles at 65 rows/pass (4x fewer PE rows than the y^T layout).
    Column 64 accumulates the softmax denominator via the ones column of v.
    8 groups pack into 2 psum banks; only the first group per 2KB zero
    region carries start=True (start wipes the whole region).
  - normalize with per-partition reciprocal + tensor_scalar (DVE), PE
    transpose back to [feat, t_q], then the output projection.

The av stage lags scores/exp by 2 units and the lag crosses p-half and
chunk boundaries, so there is no pipeline flush until the very end.
qkv(c+1) and proj(c-1) are interleaved as fillers into attention(c) so the
PE stays busy while ACT computes exp. The last chunk's projection is split
into two passes (f0 during attention, f1+add in the tail) with two
half-stores to shorten the tail. DMAs are batched via 3D access patterns.

Host re-transposes and accumulates the 4 partial [C, T] projections per batch.
"""
import numpy as np
import ml_dtypes

import concourse.bacc as bacc
import concourse.mybir as mybir
import concourse.tile as tile
from concourse.bass_utils import run_bass_kernel_spmd

F32 = mybir.dt.float32
BF16 = mybir.dt.bfloat16
EXP = mybir.ActivationFunctionType.Exp

B, T, C = 2, 2048, 1024
NH, HD = 16, 64
NCORES = 8
NG = 4            # head groups (tensor-parallel within a batch)
GC = 256          # features per group (4 heads * 64)
NFT = C // 128    # 8 feature tiles
NCH = T // 512    # 4 tq chunks

_nc_cache = {}


def build_nc():
    nc = bacc.Bacc("TRN2", target_bir_lowering=False, debug=False, num_devices=NCORES)
    xT = nc.dram_tensor("xT", [C, T], BF16, kind="ExternalInput")
    wq = nc.dram_tensor("wq", [C, GC], BF16, kind="ExternalInput")
    wk = nc.dram_tensor("wk", [C, GC], BF16, kind="ExternalInput")
    wv = nc.dram_tensor("wv", [C, GC], BF16, kind="ExternalInput")
    wpr = nc.dram_tensor("wpr", [GC, C], BF16, kind="ExternalInput")
    tri = nc.dram_tensor("tri", [128, 128], BF16, kind="ExternalInput")
    ident = nc.dram_tensor("ident", [128, 128], F32, kind="ExternalInput")
    outP = nc.dram_tensor("outP", [C, T], BF16, kind="ExternalOutput")

    xTv = xT[:].rearrange("(i p) t -> p i t", p=128)      # [128, 8, 2048]
    wqv = wq[:].rearrange("(i p) f -> p i f", p=128)      # [128, 8, 256]
    wkv = wk[:].rearrange("(i p) f -> p i f", p=128)
    wvv = wv[:].rearrange("(i p) f -> p i f", p=128)
    wprv = wpr[:].rearrange("(f p) c -> p f c", p=128)    # [128, 2, 1024]
    outv = outP[:].rearrange("(u r) t -> r u t", r=128)   # [128, 8, 2048]

    with tile.TileContext(nc) as tc:
        with (
            tc.tile_pool(name="wts", bufs=1) as wts,
            tc.tile_pool(name="xcp", bufs=1) as xcp,
            tc.tile_pool(name="qkp", bufs=1) as qkp,
            tc.tile_pool(name="vp", bufs=1) as vp,
            tc.tile_pool(name="esp", bufs=1) as esp,
            tc.tile_pool(name="ywp", bufs=1) as ywp,
            tc.tile_pool(name="outp", bufs=1) as outp,
            tc.tile_pool(name="ps", bufs=1, space="PSUM") as ps,
        ):
            tri_sb = wts.tile([128, 128], BF16, name="tri_sb")
            ident_sb = wts.tile([128, 128], F32, name="ident_sb")
            wq_sb = wts.tile([128, 2048], BF16, name="wq_sb")
            wk_sb = wts.tile([128, 2048], BF16, name="wk_sb")
            wv_sb = wts.tile([128, 2048], BF16, name="wv_sb")
            wpr_sb = wts.tile([128, 2048], BF16, name="wpr_sb")

            xc_tiles = {}

            def load_xc(c, split=False):
                xc = xcp.tile([128, 4096], BF16, tag="xc", bufs=3, name=f"xc{c}")
                xcv = xc[:].rearrange("p (i t) -> p i t", i=NFT)
                src = xTv[:, :, 512 * c : 512 * (c + 1)]
                if split:
                    # interleave per-feature-block with wq so the first qkv
                    # matmuls can start as early as possible
                    wq_sbv = wq_sb[:].rearrange("p (i f) -> p i f", i=NFT)
                    for i in range(NFT):
                        nc.sync.dma_start(wq_sbv[:, i : i + 1, :], wqv[:, i : i + 1, :])
                        nc.sync.dma_start(xcv[:, i : i + 1, :], src[:, i : i + 1, :])
                else:
                    nc.sync.dma_start(xcv, src)
                xc_tiles[c] = xc

            # ---------------- initial loads (SP queue) ----------------
            load_xc(0, split=True)
            nc.sync.dma_start(
                wk_sb[:].rearrange("p (i f) -> p i f", i=NFT), wkv
            )
            nc.sync.dma_start(
                wv_sb[:].rearrange("p (i f) -> p i f", i=NFT), wvv
            )
            nc.sync.dma_start(tri_sb[:], tri[:])
            load_xc(1)
            nc.sync.dma_start(ident_sb[:], ident[:])
            nc.sync.dma_start(
                wpr_sb[:].rearrange("p (f c) -> p f c", f=2), wprv
            )
            load_xc(2)
            load_xc(3)

            qT, kT, vb = {}, {}, {}
            ypk_tiles = {}

            def qk_group(c, kind, p, wsb, ptag):
                dst = qkp.tile(
                    [128, 512], BF16, tag="qkT", bufs=16, name=f"{kind}T{c}_{p}"
                )
                (qT if kind == "q" else kT)[(c, p)] = dst
                st = {}
                cls = []
                for i0 in range(0, NFT, 2):
                    def go(kind=kind, wsb=wsb, p=p, dst=dst, st=st, i0=i0, c=c,
                           ptag=ptag):
                        if i0 == 0:
                            st["ps"] = ps.tile(
                                [128, 512], F32, tag=ptag, bufs=2,
                                name=f"{kind}ps{c}_{p}",
                            )
                        pst = st["ps"]
                        xc = xc_tiles[c]
                        for i in (i0, i0 + 1):
                            nc.tensor.matmul(
                                pst[:],
                                wsb[:, 256 * i + 128 * p : 256 * i + 128 * (p + 1)],
                                xc[:, 512 * i : 512 * (i + 1)],
                                start=(i == 0),
                                stop=(i == NFT - 1),
                            )
                        if i0 == NFT - 2:
                            nc.vector.tensor_copy(dst[:], pst[:])
                    cls.append(go)
                return cls

            def v_group(c, ttl):
                tt = 4 * c + ttl
                dst = vp.tile([128, 260], BF16, tag="vb", bufs=16, name=f"vb{tt}")
                vb[tt] = dst
                st = {}
                cls = []
                for i0 in range(0, NFT, 2):
                    def go(ttl=ttl, tt=tt, dst=dst, st=st, i0=i0, c=c):
                        if i0 == 0:
                            st["ps"] = ps.tile(
                                [128, 256], F32, tag="mix", bufs=2, name=f"vps{tt}"
                            )
                        pst = st["ps"]
                        xc = xc_tiles[c]
                        for i in (i0, i0 + 1):
                            nc.tensor.matmul(
                                pst[:],
                                xc[:, 512 * i + 128 * ttl : 512 * i + 128 * (ttl + 1)],
                                wv_sb[:, 256 * i : 256 * (i + 1)],
                                start=(i == 0),
                                stop=(i == NFT - 1),
                            )
                        if i0 == NFT - 2:
                            dv = dst[:].rearrange("p (s c) -> p s c", s=4)
                            nc.vector.tensor_copy(
                                dv[:, :, 0:64],
                                pst[:].rearrange("p (s c) -> p s c", s=4),
                            )
                            nc.vector.memset(dv[:, :, 64:65], 1.0)
                    cls.append(go)
                return cls

            def qkv_closures(c, first=False):
                # for the pre-attention chunk 0 the sps psum slots are idle:
                # park q/k groups there so psum-slot turnaround (evac copy
                # latency) never stalls the PE
                qk_tag = "sps" if first else "mix"
                cls = []
                cls += qk_group(c, "q", 0, wq_sb, qk_tag)
                cls += qk_group(c, "k", 0, wk_sb, qk_tag)
                cls += v_group(c, 0)
                cls += qk_group(c, "q", 1, wq_sb, qk_tag)
                cls += qk_group(c, "k", 1, wk_sb, qk_tag)
                cls += v_group(c, 1)
                cls += v_group(c, 2)
                cls += v_group(c, 3)
                return cls

            def proj_closures(c, f_list=(0, 1), accumulate_sbuf=False, stores=1):
                """Projection matmuls for chunk c over wpr row-tiles f_list.
                accumulate_sbuf: add into osb instead of overwrite (2nd pass).
                stores: how many column-split store DMAs to emit (0 = none)."""
                cls = []
                if not accumulate_sbuf:
                    osb_tiles[c] = outp.tile(
                        [128, 4096], BF16, tag="osb", bufs=2, name=f"osb{c}"
                    )
                osb = osb_tiles[c]
                for u in range(NFT):
                    def go(u=u, c=c, osb=osb, f_list=f_list,
                           accumulate_sbuf=accumulate_sbuf):
                        op = ps.tile(
                            [128, 512], F32, tag="mix", bufs=2,
                            name=f"opp{c}_{u}_{f_list[0]}",
                        )
                        for n, f in enumerate(f_list):
                            nc.tensor.matmul(
                                op[:],
                                wpr_sb[:, 1024 * f + 128 * u : 1024 * f + 128 * (u + 1)],
                                ypk_tiles[c][f][:],
                                start=(n == 0),
                                stop=(n == len(f_list) - 1),
                            )
                        dst = osb[:, 512 * u : 512 * (u + 1)]
                        if accumulate_sbuf:
                            nc.vector.tensor_add(dst, dst, op[:])
                        else:
                            nc.vector.tensor_copy(dst, op[:])
                    cls.append(go)
                for s in range(stores):
                    us = NFT // stores

                    def store(c=c, osb=osb, s=s, us=us):
                        nc.sync.dma_start(
                            outv[:, us * s : us * (s + 1), 512 * c : 512 * (c + 1)],
                            osb[:].rearrange("p (u t) -> p u t", u=NFT)[
                                :, us * s : us * (s + 1), :
                            ],
                        )
                    cls.append(store)
                return cls

            osb_tiles = {}

            # ---------------- the global pipelined stream ----------------
            from collections import deque
            fillq = deque()

            def pump(k):
                while k > 0 and fillq:
                    fillq.popleft()()
                    k -= 1

            yac_tiles = {}
            ysb_tiles = {}

            def scores_exp(c, p, j):
                d = j - 4 * c
                off = 128 * max(d, 0)
                sps = ps.tile(
                    [128, 1024], F32, tag="sps", bufs=2, name=f"sps{c}{p}{j}"
                )
                jc, jj = j // 4, j % 4
                for h in range(2):
                    nc.tensor.matmul(
                        sps[:, 512 * h + off : 512 * (h + 1)],
                        kT[(jc, p)][64 * h : 64 * (h + 1), 128 * jj : 128 * (jj + 1)],
                        qT[(c, p)][64 * h : 64 * (h + 1), off:512],
                        start=True,
                        stop=True,
                    )
                es = esp.tile([128, 1024], BF16, tag="es", bufs=5, name=f"es{c}{p}{j}")
                nc.scalar.activation(
                    es[:].rearrange("p (g n) -> p g n", g=2)[:, :, off:512],
                    sps[:].rearrange("p (g n) -> p g n", g=2)[:, :, off:512],
                    EXP,
                    scale=0.125,
                )
                if d >= 0:
                    for h in range(2):
                        nc.vector.tensor_mul(
                            es[:, 512 * h + off : 512 * h + off + 128],
                            es[:, 512 * h + off : 512 * h + off + 128],
                            tri_sb[:],
                        )
                return es

            def av(c, p, j, es):
                # 8 accumulation groups packed into 2 psum banks; only the
                # first group per 2KB zero-region carries start=True (start
                # wipes the whole region; later groups see pending-zero and
                # get replace semantics on their first write). Group 7 sits
                # at col 512 (bank 1) so no group straddles banks.
                if j == 0:
                    yac_tiles[(c, p)] = ps.tile(
                        [128, 577], F32, tag="yac", bufs=1, name=f"yac{c}_{p}"
                    )
                yac = yac_tiles[(c, p)]
                d = j - 4 * c
                for q0 in range(max(d, 0), 4):
                    for h in range(2):
                        hs = 2 * p + h
                        g = 2 * q0 + h
                        base = 65 * g if g < 7 else 512
                        nc.tensor.matmul(
                            yac[:, base : base + 65],
                            es[:, 512 * h + 128 * q0 : 512 * h + 128 * (q0 + 1)],
                            vb[j][:, 65 * hs : 65 * hs + 65],
                            start=(j == 0 and g in (0, 7)),
                            stop=(j == 4 * c + q0),
                            skip_group_check=True,
                        )

            def normalize(c, p, q0):
                yac = yac_tiles[(c, p)]
                yt = ywp.tile([128, 128], F32, tag="ysb", bufs=4, name=f"ysb{c}{p}{q0}")
                for h in range(2):
                    g = 2 * q0 + h
                    base = 65 * g if g < 7 else 512
                    rc = ywp.tile(
                        [128, 1], F32, tag="rcp", bufs=8, name=f"rcp{c}{p}{q0}{h}"
                    )
                    nc.vector.reciprocal_approx_fast(
                        rc[:], yac[:, base + 64 : base + 65]
                    )
                    nc.vector.tensor_scalar_mul(
                        yt[:, 64 * h : 64 * (h + 1)],
                        yac[:, base : base + 64],
                        rc[:],
                    )
                ysb_tiles.setdefault((c, p), {})[q0] = yt

            def finish_p(c, p):
                ysb = ysb_tiles[(c, p)]
                mixT = ps.tile([128, 512], F32, tag="mix", bufs=2, name=f"yT{c}_{p}")
                for q0 in range(4):
                    # one psum group for all 4 transposes: start=True would
                    # wipe the shared 2KB zero-region of the earlier ones
                    nc.tensor.matmul(
                        mixT[:, 128 * q0 : 128 * (q0 + 1)], ysb[q0][:], ident_sb[:],
                        is_transpose=True, start=(q0 == 0), stop=(q0 == 3),
                        skip_group_check=True,
                    )
                yp = ywp.tile([128, 512], BF16, tag="ypk", bufs=4, name=f"ypk{c}_{p}")
                nc.vector.tensor_copy(yp[:], mixT[:])
                ypk_tiles.setdefault(c, {})[p] = yp
                if c == NCH - 1 and p == 0:
                    # last chunk: run the f=0 projection pass as fillers while
                    # p=1 attention runs; f=1 + add + stores happen in the tail
                    fillq.extend(
                        proj_closures(c, f_list=(0,), accumulate_sbuf=False, stores=0)
                    )
                elif p == 1 and c < NCH - 1:
                    # chunk c's projection becomes available once both ypk
                    # halves exist; queue it as fillers for the next chunk
                    fillq.extend(proj_closures(c))

            def step(unit):
                c, p, j, es = unit
                av(c, p, j, es)
                if j >= 4 * c:
                    normalize(c, p, j - 4 * c)
                if j == 4 * c + 3:
                    finish_p(c, p)

            esq = []
            for cl in qkv_closures(0, first=True):
                cl()
            for c in range(NCH):
                if c + 1 < NCH:
                    fillq.extend(qkv_closures(c + 1))
                units = [(c, p, j) for p in range(2) for j in range(4 * c + 4)]
                for idx, (cc, p, j) in enumerate(units):
                    esq.append((cc, p, j, scores_exp(cc, p, j)))
                    remaining = len(units) - idx
                    pump(-(-len(fillq) // remaining))
                    if len(esq) > 2:
                        step(esq.pop(0))
                pump(len(fillq))
            while esq:
                step(esq.pop(0))
            # tail: last chunk's second projection pass + half stores
            for cl in proj_closures(NCH - 1, f_list=(1,), accumulate_sbuf=True,
                                    stores=2):
                cl()

    nc.compile()
    return nc


def _get_nc():
    if "nc" not in _nc_cache:
        _nc_cache["nc"] = build_nc()
    return _nc_cache["nc"]


def _in_maps(x, W_attn, W_proj):
    bf = ml_dtypes.bfloat16
    tri = np.triu(np.ones((128, 128), np.float32)).astype(bf)
    ident = np.eye(128, dtype=np.float32)
    maps = []
    for core in range(NCORES):
        b, g = core // NG, core % NG
        lo = g * GC
        maps.append(
            {
                "xT": np.ascontiguousarray(x[b].T).astype(bf),
                "wq": np.ascontiguousarray(W_attn[:, lo : lo + GC]).astype(bf),
                "wk": np.ascontiguousarray(W_attn[:, C + lo : C + lo + GC]).astype(bf),
                "wv": np.ascontiguousarray(W_attn[:, 2 * C + lo : 2 * C + lo + GC]).astype(bf),
                "wpr": np.ascontiguousarray(W_proj[lo : lo + GC, :]).astype(bf),
                "tri": tri,
                "ident": ident,
            }
        )
    return maps


def kernel(x, W_attn, W_proj, **run_kwargs):
    x = np.asarray(x, np.float32)
    W_attn = np.asarray(W_attn, np.float32)
    W_proj = np.asarray(W_proj, np.float32)
    nc = _get_nc()
    res = run_bass_kernel_spmd(
        nc, _in_maps(x, W_attn, W_proj), core_ids=list(range(NCORES)), **run_kwargs
    )
    out = np.empty((B, T, C), np.float32)
    for b in range(B):
        acc = res.results[NG * b]["outP"].astype(np.float32)
        for g in range(1, NG):
            acc += res.results[NG * b + g]["outP"].astype(np.float32)
        out[b] = acc.T
    if run_kwargs:
        kernel.last_result = res
    return out


# revision 30
# speedup vs baseline: 1.3360x; 1.0937x over previous
"""Causal self-attention (B=2, T=2048, C=1024, 16 heads) on 8 trn2 cores.

Sharding: 2 batches x 4 head-groups (4 heads each per core). All matmul
operands bf16 (host-converted); psum f32.

Per core, a single software-pipelined stream over (chunk, p-half, key-tile)
units:
  - qkv projection per 512-query chunk (q/k transposed [feat, t] per-chunk
    tiles, v in [t, feat+ones] layout for the denominator trick).
  - scores kept transposed [t_k, t_q] in psum, exp on ACT -> bf16 sbuf,
    causal diagonal masked via tri multiply (DVE).
  - av uses es as the *stationary* operand: out y[t_q, 65] accumulates over
    t_k tiles at 65 rows/pass (4x fewer PE rows than the y^T layout).
    Column 64 accumulates the softmax denominator via the ones column of v.
    8 groups pack into 2 psum banks; only the first group per 2KB zero
    region carries start=True (start wipes the whole region).
  - normalize with per-partition reciprocal + tensor_scalar (DVE), PE
    transpose back to [feat, t_q], then the output projection.

The av stage lags scores/exp by 2 units and the lag crosses p-half and
chunk boundaries, so there is no pipeline flush until the very end.
qkv(c+1) and proj(c-1) are interleaved as fillers into attention(c) so the
PE stays busy while ACT computes exp. The last chunk's projection is split
into two passes (f0 during attention, f1+add in the tail) with two
half-stores to shorten the tail. DMAs are batched via 3D access patterns.

Host re-transposes and accumulates the 4 partial [C, T] projections per batch.
"""
import numpy as np
import ml_dtypes

import concourse.bacc as bacc
import concourse.mybir as mybir
import concourse.tile as tile
from concourse.bass_utils import run_bass_kernel_spmd

F32 = mybir.dt.float32
BF16 = mybir.dt.bfloat16
EXP = mybir.ActivationFunctionType.Exp

B, T, C = 2, 2048, 1024
NH, HD = 16, 64
NCORES = 8
NG = 4            # head groups (tensor-parallel within a batch)
GC = 256          # features per group (4 heads * 64)
NFT = C // 128    # 8 feature tiles
NCH = T // 512    # 4 tq chunks

_nc_cache = {}


def build_nc():
    nc = bacc.Bacc("TRN2", target_bir_lowering=False, debug=False, num_devices=NCORES)
    xT = nc.dram_tensor("xT", [C, T], BF16, kind="ExternalInput")
    wq = nc.dram_tensor("wq", [C, GC], BF16, kind="ExternalInput")
    wk = nc.dram_tensor("wk", [C, GC], BF16, kind="ExternalInput")
    wv = nc.dram_tensor("wv", [C, GC], BF16, kind="ExternalInput")
    wpr = nc.dram_tensor("wpr", [GC, C], BF16, kind="ExternalInput")
    tri = nc.dram_tensor("tri", [128, 128], BF16, kind="ExternalInput")
    ident = nc.dram_tensor("ident", [128, 128], F32, kind="ExternalInput")
    outP = nc.dram_tensor("outP", [C, T], BF16, kind="ExternalOutput")

    xTv = xT[:].rearrange("(i p) t -> p i t", p=128)      # [128, 8, 2048]
    wqv = wq[:].rearrange("(i p) f -> p i f", p=128)      # [128, 8, 256]
    wkv = wk[:].rearrange("(i p) f -> p i f", p=128)
    wvv = wv[:].rearrange("(i p) f -> p i f", p=128)
    wprv = wpr[:].rearrange("(f p) c -> p f c", p=128)    # [128, 2, 1024]
    outv = outP[:].rearrange("(u r) t -> r u t", r=128)   # [128, 8, 2048]

    with tile.TileContext(nc) as tc:
        with (
            tc.tile_pool(name="wts", bufs=1) as wts,
            tc.tile_pool(name="xcp", bufs=1) as xcp,
            tc.tile_pool(name="qkp", bufs=1) as qkp,
            tc.tile_pool(name="vp", bufs=1) as vp,
            tc.tile_pool(name="esp", bufs=1) as esp,
            tc.tile_pool(name="ywp", bufs=1) as ywp,
            tc.tile_pool(name="outp", bufs=1) as outp,
            tc.tile_pool(name="ps", bufs=1, space="PSUM") as ps,
        ):
            tri_sb = wts.tile([128, 128], BF16, name="tri_sb")
            ident_sb = wts.tile([128, 128], F32, name="ident_sb")
            identb_sb = wts.tile([128, 128], BF16, name="identb_sb")
            wq_sb = wts.tile([128, 2048], BF16, name="wq_sb")
            wk_sb = wts.tile([128, 2048], BF16, name="wk_sb")
            wv_sb = wts.tile([128, 2048], BF16, name="wv_sb")
            wpr_sb = wts.tile([128, 2048], BF16, name="wpr_sb")

            xc_tiles = {}

            def load_xc(c, split=False):
                xc = xcp.tile([128, 4096], BF16, tag="xc", bufs=3, name=f"xc{c}")
                xcv = xc[:].rearrange("p (i t) -> p i t", i=NFT)
                src = xTv[:, :, 512 * c : 512 * (c + 1)]
                if split:
                    # interleave small pieces with wq so the first qkv
                    # matmuls start early and HWDGE (~630ns/DMA) keeps up
                    wq_sbv = wq_sb[:].rearrange("p (i f) -> p i f", i=NFT)
                    nc.sync.dma_start(wq_sbv[:, 0:2, :], wqv[:, 0:2, :])
                    nc.sync.dma_start(xcv[:, 0:1, :], src[:, 0:1, :])
                    nc.sync.dma_start(xcv[:, 1:2, :], src[:, 1:2, :])
                    nc.sync.dma_start(wq_sbv[:, 2:4, :], wqv[:, 2:4, :])
                    nc.sync.dma_start(xcv[:, 2:4, :], src[:, 2:4, :])
                    nc.sync.dma_start(wq_sbv[:, 4:8, :], wqv[:, 4:8, :])
                    nc.sync.dma_start(xcv[:, 4:6, :], src[:, 4:6, :])
                    nc.sync.dma_start(
                        wk_sb[:].rearrange("p (i f) -> p i f", i=NFT), wkv
                    )
                    nc.sync.dma_start(xcv[:, 6:8, :], src[:, 6:8, :])
                else:
                    nc.sync.dma_start(xcv, src)
                xc_tiles[c] = xc

            # ---------------- initial loads (SP queue) ----------------
            load_xc(0, split=True)
            nc.sync.dma_start(
                wv_sb[:].rearrange("p (i f) -> p i f", i=NFT), wvv
            )
            nc.sync.dma_start(tri_sb[:], tri[:])
            load_xc(1)
            nc.sync.dma_start(ident_sb[:], ident[:])
            nc.vector.tensor_copy(identb_sb[:], ident_sb[:])
            nc.sync.dma_start(
                wpr_sb[:].rearrange("p (f c) -> p f c", f=2), wprv
            )
            load_xc(2)
            load_xc(3)

            qT, kT, vb = {}, {}, {}
            ypk_tiles = {}

            def qk_group(c, kind, p, wsb, ptag):
                dst = qkp.tile(
                    [128, 512], BF16, tag="qkT", bufs=16, name=f"{kind}T{c}_{p}"
                )
                (qT if kind == "q" else kT)[(c, p)] = dst
                st = {}
                cls = []
                for i0 in range(0, NFT, 2):
                    def go(kind=kind, wsb=wsb, p=p, dst=dst, st=st, i0=i0, c=c,
                           ptag=ptag):
                        if i0 == 0:
                            st["ps"] = ps.tile(
                                [128, 512], F32, tag=ptag, bufs=2,
                                name=f"{kind}ps{c}_{p}",
                            )
                        pst = st["ps"]
                        xc = xc_tiles[c]
                        for i in (i0, i0 + 1):
                            nc.tensor.matmul(
                                pst[:],
                                wsb[:, 256 * i + 128 * p : 256 * i + 128 * (p + 1)],
                                xc[:, 512 * i : 512 * (i + 1)],
                                start=(i == 0),
                                stop=(i == NFT - 1),
                            )
                        if i0 == NFT - 2:
                            nc.vector.tensor_copy(dst[:], pst[:])
                    cls.append(go)
                return cls

            def v_group(c, ttl):
                tt = 4 * c + ttl
                dst = vp.tile([128, 260], BF16, tag="vb", bufs=16, name=f"vb{tt}")
                vb[tt] = dst
                st = {}
                cls = []
                for i0 in range(0, NFT, 2):
                    def go(ttl=ttl, tt=tt, dst=dst, st=st, i0=i0, c=c):
                        if i0 == 0:
                            st["ps"] = ps.tile(
                                [128, 256], F32, tag="mix", bufs=2, name=f"vps{tt}"
                            )
                        pst = st["ps"]
                        xc = xc_tiles[c]
                        for i in (i0, i0 + 1):
                            nc.tensor.matmul(
                                pst[:],
                                xc[:, 512 * i + 128 * ttl : 512 * i + 128 * (ttl + 1)],
                                wv_sb[:, 256 * i : 256 * (i + 1)],
                                start=(i == 0),
                                stop=(i == NFT - 1),
                            )
                        if i0 == NFT - 2:
                            dv = dst[:].rearrange("p (s c) -> p s c", s=4)
                            nc.vector.tensor_copy(
                                dv[:, :, 0:64],
                                pst[:].rearrange("p (s c) -> p s c", s=4),
                            )
                            nc.vector.memset(dv[:, :, 64:65], 1.0)
                    cls.append(go)
                return cls

            def qkv_closures(c, first=False):
                # for the pre-attention chunk 0 the sps psum slots are idle:
                # park q/k groups there so psum-slot turnaround (evac copy
                # latency) never stalls the PE.  p1's q/k are only needed
                # half a chunk later, so they go last (the pacer pushes them
                # toward the ACT-bound end of the window).
                qk_tag = "sps" if first else "mix"
                cls = []
                cls += qk_group(c, "q", 0, wq_sb, qk_tag)
                cls += qk_group(c, "k", 0, wk_sb, qk_tag)
                cls += v_group(c, 0)
                cls += v_group(c, 1)
                cls += v_group(c, 2)
                cls += v_group(c, 3)
                cls += qk_group(c, "q", 1, wq_sb, qk_tag)
                cls += qk_group(c, "k", 1, wk_sb, qk_tag)
                return cls

            def proj_closures(c, f_list=(0, 1), inject=False, stores=1):
                """Projection matmuls for chunk c over wpr row-tiles f_list.
                inject: re-feed the existing osb through the PE (identity
                matmul) so the f_list pass accumulates onto it in psum and
                the evacuation stays a plain copy (alternated DVE/ACT).
                stores: how many column-split store DMAs to emit (0 = none)."""
                cls = []
                if not inject:
                    osb_tiles[c] = outp.tile(
                        [128, 4096], BF16, tag="osb", bufs=3, name=f"osb{c}"
                    )
                osb = osb_tiles[c]
                for u in range(NFT):
                    def go(u=u, c=c, osb=osb, f_list=f_list, inject=inject):
                        # in the tail (inject pass) the scores psum slots are
                        # idle: alternate tags for 4-deep slot rotation
                        tag = "sps" if (inject and u % 2) else "mix"
                        op = ps.tile(
                            [128, 512], F32, tag=tag, bufs=2,
                            name=f"opp{c}_{u}_{f_list[0]}",
                        )
                        dst = osb[:, 512 * u : 512 * (u + 1)]
                        if inject:
                            nc.tensor.matmul(
                                op[:], identb_sb[:], dst, start=True, stop=False
                            )
                        for n, f in enumerate(f_list):
                            nc.tensor.matmul(
                                op[:],
                                wpr_sb[:, 1024 * f + 128 * u : 1024 * f + 128 * (u + 1)],
                                ypk_tiles[c][f][:],
                                start=(n == 0 and not inject),
                                stop=(n == len(f_list) - 1),
                            )
                        if inject and u % 2 == 0:
                            nc.scalar.copy(dst, op[:])
                        else:
                            nc.vector.tensor_copy(dst, op[:])
                    cls.append(go)
                def mk_store(s, us):
                    def store(c=c, osb=osb, s=s, us=us):
                        nc.sync.dma_start(
                            outv[:, us * s : us * (s + 1), 512 * c : 512 * (c + 1)],
                            osb[:].rearrange("p (u t) -> p u t", u=NFT)[
                                :, us * s : us * (s + 1), :
                            ],
                        )
                    return store

                if stores:
                    us = NFT // stores
                    if inject:
                        # interleave each partial store right after its last
                        # u-evacuation so the final store starts early
                        out_cls = []
                        for u in range(NFT):
                            out_cls.append(cls[u])
                            if (u + 1) % us == 0:
                                out_cls.append(mk_store((u + 1) // us - 1, us))
                        cls = out_cls
                    else:
                        for s in range(stores):
                            cls.append(mk_store(s, us))
                return cls

            osb_tiles = {}

            # ---------------- the global pipelined stream ----------------
            from collections import deque
            fillq = deque()
            yac_tiles = {}
            ysb_tiles = {}

            def scores_exp(c, p, j):
                d = j - 4 * c
                off = 128 * max(d, 0)
                sps = ps.tile(
                    [128, 1024], F32, tag="sps", bufs=2, name=f"sps{c}{p}{j}"
                )
                jc, jj = j // 4, j % 4
                for h in range(2):
                    nc.tensor.matmul(
                        sps[:, 512 * h + off : 512 * (h + 1)],
                        kT[(jc, p)][64 * h : 64 * (h + 1), 128 * jj : 128 * (jj + 1)],
                        qT[(c, p)][64 * h : 64 * (h + 1), off:512],
                        start=True,
                        stop=True,
                    )
                es = esp.tile([128, 1024], BF16, tag="es", bufs=5, name=f"es{c}{p}{j}")
                nc.scalar.activation(
                    es[:].rearrange("p (g n) -> p g n", g=2)[:, :, off:512],
                    sps[:].rearrange("p (g n) -> p g n", g=2)[:, :, off:512],
                    EXP,
                    scale=0.125,
                )
                if d >= 0:
                    for h in range(2):
                        nc.vector.tensor_mul(
                            es[:, 512 * h + off : 512 * h + off + 128],
                            es[:, 512 * h + off : 512 * h + off + 128],
                            tri_sb[:],
                        )
                return es

            def av(c, p, j, es):
                # 8 accumulation groups packed into 2 psum banks; only the
                # first group per 2KB zero-region carries start=True (start
                # wipes the whole region; later groups see pending-zero and
                # get replace semantics on their first write). Group 7 sits
                # at col 512 (bank 1) so no group straddles banks.
                if j == 0:
                    yac_tiles[(c, p)] = ps.tile(
                        [128, 577], F32, tag="yac", bufs=1, name=f"yac{c}_{p}"
                    )
                yac = yac_tiles[(c, p)]
                d = j - 4 * c
                for q0 in range(max(d, 0), 4):
                    for h in range(2):
                        hs = 2 * p + h
                        g = 2 * q0 + h
                        base = 65 * g if g < 7 else 512
                        nc.tensor.matmul(
                            yac[:, base : base + 65],
                            es[:, 512 * h + 128 * q0 : 512 * h + 128 * (q0 + 1)],
                            vb[j][:, 65 * hs : 65 * hs + 65],
                            start=(j == 0 and g in (0, 7)),
                            stop=(j == 4 * c + q0),
                            skip_group_check=True,
                        )

            def normalize(c, p, q0):
                yac = yac_tiles[(c, p)]
                yt = ywp.tile([128, 128], BF16, tag="ysb", bufs=4, name=f"ysb{c}{p}{q0}")
                for h in range(2):
                    g = 2 * q0 + h
                    base = 65 * g if g < 7 else 512
                    rc = ywp.tile(
                        [128, 1], F32, tag="rcp", bufs=8, name=f"rcp{c}{p}{q0}{h}"
                    )
                    nc.vector.reciprocal_approx_fast(
                        rc[:], yac[:, base + 64 : base + 65]
                    )
                    nc.vector.tensor_scalar_mul(
                        yt[:, 64 * h : 64 * (h + 1)],
                        yac[:, base : base + 64],
                        rc[:],
                    )
                ysb_tiles.setdefault((c, p), {})[q0] = yt

            def finish_p(c, p):
                ysb = ysb_tiles[(c, p)]
                mixT = ps.tile([128, 512], BF16, tag="mix", bufs=2, name=f"yT{c}_{p}")
                for q0 in range(4):
                    # one psum group for all 4 transposes: start=True would
                    # wipe the shared 2KB zero-region of the earlier ones
                    nc.tensor.matmul(
                        mixT[:, 128 * q0 : 128 * (q0 + 1)], ysb[q0][:], identb_sb[:],
                        is_transpose=True, start=(q0 == 0), stop=(q0 == 3),
                        skip_group_check=True,
                    )
                yp = ywp.tile([128, 512], BF16, tag="ypk", bufs=6, name=f"ypk{c}_{p}")
                nc.vector.tensor_copy(yp[:], mixT[:])
                ypk_tiles.setdefault(c, {})[p] = yp
                if c == NCH - 1 and p == 0:
                    # last chunk: run the f=0 projection pass as fillers while
                    # p=1 attention runs; f=1 + add + stores happen in the tail
                    fillq.extend(
                        proj_closures(c, f_list=(0,), stores=0)
                    )
                elif p == 1 and c < NCH - 1:
                    # chunk c's projection becomes available once both ypk
                    # halves exist; defer it into the latest ACT-bound window
                    # that still has PE idle (c0 -> chunk 2, c1/c2 -> chunk 3)
                    tgt = min(c + 2, NCH - 1)
                    if tgt <= cur_chunk[0]:
                        fillq.extend(proj_closures(c))
                    else:
                        scheduled.setdefault(tgt, []).extend(proj_closures(c))

            def step(unit):
                c, p, j, es = unit
                av(c, p, j, es)
                if j >= 4 * c:
                    normalize(c, p, j - 4 * c)
                if j == 4 * c + 3:
                    finish_p(c, p)

            esq = []
            scheduled = {}
            cur_chunk = [0]
            for cl in qkv_closures(0, first=True):
                cl()
            emitted = [0]

            def pump_n(k):
                while k > 0 and fillq:
                    fillq.popleft()()
                    emitted[0] += 1
                    k -= 1

            for c in range(NCH):
                cur_chunk[0] = c
                fillq.extend(scheduled.pop(c, []))
                if c + 1 < NCH:
                    fillq.extend(qkv_closures(c + 1))
                units = [(c, p, j) for p in range(2) for j in range(4 * c + 4)]
                base = emitted[0]
                for idx, (cc, p, j) in enumerate(units):
                    esq.append((cc, p, j, scores_exp(cc, p, j)))
                    # evenly pace the currently-known filler backlog over the
                    # rest of this chunk's units (new arrivals re-pace)
                    total = emitted[0] + len(fillq)
                    target = base + (total - base) * (idx + 1) // len(units)
                    pump_n(target - emitted[0])
                    if len(esq) > 3:
                        step(esq.pop(0))
                pump_n(len(fillq))
            while esq:
                step(esq.pop(0))
            # tail: last chunk's f=1 projection pass accumulates onto the
            # f=0 result re-fed through the PE; plain copies evacuate
            # (alternating ACT/DVE) and two half-stores finish
            for cl in proj_closures(NCH - 1, f_list=(1,), inject=True, stores=4):
                cl()

    nc.compile()
    return nc


def _get_nc():
    if "nc" not in _nc_cache:
        _nc_cache["nc"] = build_nc()
    return _nc_cache["nc"]


def _in_maps(x, W_attn, W_proj):
    bf = ml_dtypes.bfloat16
    tri = np.triu(np.ones((128, 128), np.float32)).astype(bf)
    ident = np.eye(128, dtype=np.float32)
    maps = []
    for core in range(NCORES):
        b, g = core // NG, core % NG
        lo = g * GC
        maps.append(
            {
                "xT": np.ascontiguousarray(x[b].T).astype(bf),
                "wq": np.ascontiguousarray(W_attn[:, lo : lo + GC]).astype(bf),
                "wk": np.ascontiguousarray(W_attn[:, C + lo : C + lo + GC]).astype(bf),
                "wv": np.ascontiguousarray(W_attn[:, 2 * C + lo : 2 * C + lo + GC]).astype(bf),
                "wpr": np.ascontiguousarray(W_proj[lo : lo + GC, :]).astype(bf),
                "tri": tri,
                "ident": ident,
            }
        )
    return maps


def kernel(x, W_attn, W_proj, **run_kwargs):
    x = np.asarray(x, np.float32)
    W_attn = np.asarray(W_attn, np.float32)
    W_proj = np.asarray(W_proj, np.float32)
    nc = _get_nc()
    res = run_bass_kernel_spmd(
        nc, _in_maps(x, W_attn, W_proj), core_ids=list(range(NCORES)), **run_kwargs
    )
    out = np.empty((B, T, C), np.float32)
    for b in range(B):
        acc = res.results[NG * b]["outP"].astype(np.float32)
        for g in range(1, NG):
            acc += res.results[NG * b + g]["outP"].astype(np.float32)
        out[b] = acc.T
    if run_kwargs:
        kernel.last_result = res
    return out
